# revision 22
# baseline (speedup 1.0000x reference)
"""Trainium2 Bass kernel: 3x3 VALID conv (NCHW/OIHW) + bias + /2 + LeakyReLU.

Full-input contract: kernel(x, weight, bias) takes the complete arrays,
shards the batch dim across 8 NeuronCores (2 images per core), runs the
Bass program SPMD, and concatenates the per-core outputs.

Compute strategy (per core, per image):
  - SBUF layout: input row h, channel c -> partition 32*(h%4)+c, free
    offset (h//4)*258 + w  (rows padded 256->258 so the kw=1,2 taps can
    read a full 256-wide window without crossing rows).
  - Each output row o needs input rows o..o+2, which land in 3 distinct
    32-partition groups -> the 3 kh-taps run as concurrent 32x32 PE
    sub-tiles (tile_position row groups). 4 output rows are processed per
    round in the 4 PSUM column groups -> 12 concurrent sub-tiles.
  - kw taps are free-dim offsets (0/1/2) into the same SBUF row.
  - bf16 compute; the SWDGE input DMAs cast f32->bf16 in flight (free).
  - Each kh tap accumulates in its own PSUM plane (a region may only be
    written by one tile position); planes rotate over all 8 PSUM banks
    for eviction-chain pipelining. Eviction: ACT copy + 2 DVE adds +
    one ScalarE Lrelu (out = Lrelu(sum*0.5 + b/2), alpha=0.01) into an
    SBUF staging tile DMA'd out in 32-row batches.
"""

import sys

if "/opt/trn_rl_repo" not in sys.path:
    sys.path.insert(0, "/opt/trn_rl_repo")

import numpy as np

import concourse.bass as bass
import concourse.tile as tile
from concourse import bacc
from concourse import mybir
from concourse.bass_utils import run_bass_kernel_spmd

N_CORES = 8
IMGS_PER_CORE = 2
C = 32
H = 256
W = 256
OH = 254
OW = 254
G = 4            # partition groups = h mod 4
HD = H // G      # 64 rows per group
WPAD = W + 2     # per-row pad so kw shifts stay in-row
NFREE = 256      # matmul free dim (>=256 keeps float32r at full rate)
F32 = mybir.dt.float32
F32R = mybir.dt.float32r
BF16 = mybir.dt.bfloat16
LRELU = mybir.ActivationFunctionType.Lrelu


def build_nc(repeat=1):
    nc = bacc.Bacc()
    x_ext = nc.declare_dram_parameter(
        "x", [IMGS_PER_CORE, C, H, W], F32, isOutput=False
    )
    # host-prepared: wr[32g+k, tap, m] = weight[m, k, kh, kw]; biasr = bias/2 tiled 4x
    w_ext = nc.declare_dram_parameter("wr", [128, 9, C], BF16, isOutput=False)
    b_ext = nc.declare_dram_parameter("biasr", [128], F32, isOutput=False)
    y_ext = nc.declare_dram_parameter(
        "y", [IMGS_PER_CORE, C, OH, OW], F32, isOutput=True
    )

    with tile.TileContext(nc) as tc:
        with (
            tc.tile_pool(name="xp", bufs=2) as xpool,
            tc.tile_pool(name="const", bufs=1) as cpool,
            tc.tile_pool(name="ps", bufs=1, space="PSUM") as pspool,
            tc.tile_pool(name="ev", bufs=6) as evpool,
            tc.tile_pool(name="outp", bufs=3) as opool,
        ):
            # Weights: partition 32g+k (k = c_in), free (tap, m = c_out),
            # replicated into all 4 partition groups so lhsT.base_partition
            # matches the rhs row group (tile_position auto-derivation).
            w_sb = cpool.tile([128, 9, C], BF16)
            nc.sync.dma_start(out=w_sb, in_=w_ext[:])

            bias_half = cpool.tile([128, 1], F32)
            nc.sync.dma_start(out=bias_half, in_=b_ext[:].unsqueeze(1))


            bank_ctr = [0]
            for img_rep in range(IMGS_PER_CORE * repeat):
                img = img_rep % IMGS_PER_CORE
                x_sb = xpool.tile([128, HD, WPAD], BF16)
                nc.vector.memset(x_sb[:, :, W:WPAD], 0.0)
                # h = hd*4 + hm  ->  partition group hm, free row hd
                # SWDGE dma casts f32 -> bf16 in flight
                xsrc = x_ext[:][img].rearrange("c (hd hm) w -> hm c hd w", hm=G)
                # halves let round 0 start after ~4MB instead of 8MB
                for half in range(2):
                    hd0, hd1 = 32 * half, 32 * (half + 1)
                    for g in range(G):
                        nc.gpsimd.dma_start(
                            out=x_sb[32 * g : 32 * (g + 1), hd0:hd1, 0:W],
                            in_=xsrc[g][:, hd0:hd1, :],
                        )

                for b in range(8):  # batches of up to 32 output rows
                    rows0 = 32 * b
                    nrounds = min(8, (OH - rows0 + 3) // 4)
                    stage = opool.tile([128, 8, NFREE], F32)
                    for rb in range(nrounds):
                        h0 = rows0 + 4 * rb
                        njs = min(4, OH - h0)
                        # one PSUM plane per kh: each [32,256] region is
                        # written by exactly one PE tile position (multi-
                        # row-group accumulation into one region faults).
                        # rotate the 3 planes across all 8 PSUM banks for
                        # ~2.7 rounds of eviction-chain pipelining.
                        c0 = bank_ctr[0]
                        bank_ctr[0] += 3
                        pl0 = pspool.tile([128, NFREE], F32, tag=f"bk{c0 % 8}")
                        pl1 = pspool.tile(
                            [128, NFREE], F32, tag=f"bk{(c0 + 1) % 8}"
                        )
                        pl2 = pspool.tile(
                            [128, NFREE], F32, tag=f"bk{(c0 + 2) % 8}"
                        )
                        planes = [pl0, pl1, pl2]
                        for j in range(njs):
                            o = h0 + j
                            for kh in range(3):
                                rho = o + kh
                                g = rho % 4
                                hd = rho // 4
                                for kw in range(3):
                                    nc.tensor.matmul(
                                        planes[kh][32 * j : 32 * (j + 1), :],
                                        w_sb[
                                            32 * g : 32 * (g + 1),
                                            kh * 3 + kw,
                                            :,
                                        ],
                                        x_sb[
                                            32 * g : 32 * (g + 1),
                                            hd,
                                            kw : kw + NFREE,
                                        ],
                                        start=(kw == 0),
                                        stop=(kw == 2),
                                        tile_position=(32 * g, 32 * j),
                                    )
                        np_used = 32 * njs
                        a_sb = evpool.tile([128, NFREE], F32, tag="a")
                        a2_sb = evpool.tile([128, NFREE], F32, tag="a2")
                        b_sb = evpool.tile([128, NFREE], F32, tag="b")
                        nc.scalar.activation(
                            out=a_sb[0:np_used],
                            in_=pl0[0:np_used],
                            func=mybir.ActivationFunctionType.Copy,
                            bias=0.0,
                            scale=1.0,
                        )
                        nc.vector.tensor_add(
                            a2_sb[0:np_used], a_sb[0:np_used], pl1[0:np_used]
                        )
                        nc.vector.tensor_add(
                            b_sb[0:np_used], a2_sb[0:np_used], pl2[0:np_used]
                        )
                        nc.scalar.activation(
                            out=stage[0:np_used, rb, :],
                            in_=b_sb[0:np_used],
                            func=LRELU,
                            bias=bias_half[0:np_used],
                            scale=0.5,
                            alpha=0.01,
                        )
                    # store: per column group j, rows rows0+4*rb+j (stride 4)
                    if True:
                        for j in range(4):
                            nrb_j = 0
                            while nrb_j < nrounds and rows0 + 4 * nrb_j + j < OH:
                                nrb_j += 1
                            if nrb_j == 0:
                                continue
                            src = stage[32 * j : 32 * (j + 1), 0:nrb_j, 0:OW]
                            dst = y_ext[:][img][
                                :,
                                rows0 + j : min(rows0 + j + 4 * nrb_j, OH) : 4,
                                :,
                            ]
                            nc.sync.dma_start(out=dst, in_=src)
    nc.compile()
    return nc


_CACHE = {}


def _get_nc(repeat=1):
    key = f"nc{repeat}"
    if key not in _CACHE:
        _CACHE[key] = build_nc(repeat)
    return _CACHE[key]


def kernel(x, weight, bias):
    x = np.ascontiguousarray(np.asarray(x, dtype=np.float32))
    weight = np.asarray(weight, dtype=np.float32)
    bias = np.asarray(bias, dtype=np.float32)
    # wr[32g+k, tap, m] = weight[m, k, kh, kw], replicated into 4 groups
    import ml_dtypes
    wr = np.ascontiguousarray(
        np.tile(
            np.transpose(weight, (1, 2, 3, 0)).reshape(C, 9, C), (G, 1, 1)
        ).astype(ml_dtypes.bfloat16)
    )
    biasr = np.ascontiguousarray(np.tile(bias * 0.5, G))
    nc = _get_nc()
    in_maps = [
        {
            "x": x[IMGS_PER_CORE * i : IMGS_PER_CORE * (i + 1)],
            "wr": wr,
            "biasr": biasr,
        }
        for i in range(N_CORES)
    ]
    res = run_bass_kernel_spmd(nc, in_maps, core_ids=list(range(N_CORES)))
    return np.concatenate([res.results[i]["y"] for i in range(N_CORES)], axis=0)
